# revision 13
# baseline (speedup 1.0000x reference)
"""Trainium2 Bass kernel for the hypergraph-conv survival model.

Sharding: graph/data parallel over 8 NeuronCores. Core k owns graphs
2k,2k+1 (a contiguous node range, since `batch` is sorted). Both hconv
message-passing directions are sharded by that node range:
  phase A: gather local z rows by incidence sorted by hyperedge;
           segment-sum via one-hot bf16 matmuls into 128-wide hyperedge
           windows (one-hot is the stationary operand, so PSUM comes out
           hyperedge-major and needs no transpose). B^-1 is folded into
           the one-hot values per edge.
  AllReduce the [25088,128] bf16 partial hyperedge table (in 2 chunks,
           the first overlapping the tail of phase A).
  phase B: gather reduced hyperedge rows by incidence sorted by node;
           segment-sum into 256-wide node windows feature-major. D^-1 is
           folded into the one-hot; bias+relu runs on the Act engine.
Weights are replicated; the final MLP runs per-core on its two pooled
graph rows, all in fp32.
"""

import os
import sys

sys.path.insert(0, "/opt/trn_rl_repo")

import numpy as np

KDEBUG = bool(int(os.environ.get("KDEBUG", "0")))
PREP_GATHER = bool(int(os.environ.get("KPREP", "1")))

# ---- static problem sizes (from the reference) ----
N = 100_000
E = 800_000
M = 25_000
B_GRAPHS = 16
F_IN = 64
H = 128
EPS = 1e-5
NCORE = 8

NK_PAD = 13312          # padded per-core node count (104*128; 52 windows of 256)
M_PAD = 25088           # padded hyperedge count  (196*128)

WA = 128                # phase A window width (hyperedges)
NWA = M_PAD // WA       # 196 hedge windows
TW_A = 5                # tiles (128 edges) per hedge window (max seen 593)
NTA = NWA * TW_A        # 980 tiles
E_PAD_A = NTA * 128     # 125440
CHW_A = 14              # windows per gather chunk -> 14 chunks
NCH_A = NWA // CHW_A
NI_A = CHW_A * TW_A * 128   # idxs per phase-A gather (8960)

WB = 256                # phase B window width (nodes)
NWB = NK_PAD // WB      # 52 node windows
TW_B = 18               # tiles per node window (max seen 2201)
NTB = NWB * TW_B        # 936 tiles
E_PAD_B = NTB * 128     # 119808
CHW_B = 4               # windows per gather chunk -> 13 chunks
NCH_B = NWB // CHW_B
NI_B = CHW_B * TW_B * 128   # idxs per phase-B gather (9216)

AR_SPLIT_W = 98         # phase-A windows in the first AllReduce chunk
AR_SPLIT = AR_SPLIT_W * WA  # 12544 rows

_COMPILED = None


def _build_nc():
    import concourse.bacc as bacc
    import concourse.mybir as mybir
    from concourse.tile import TileContext
    from concourse import library_config

    f32 = mybir.dt.float32
    bf16 = mybir.dt.bfloat16
    i16 = mybir.dt.int16
    i32 = mybir.dt.int32
    EQ = mybir.AluOpType.is_equal
    ADD = mybir.AluOpType.add
    MULT = mybir.AluOpType.mult
    RELU = mybir.ActivationFunctionType.Relu

    nc = bacc.Bacc("TRN2", target_bir_lowering=False, num_devices=NCORE)

    def inp(name, shape, dt=f32):
        return nc.dram_tensor(name, shape, dt, kind="ExternalInput")

    xT = inp("xT", [F_IN, NK_PAD], bf16)
    idxA = inp("idxA", [128, E_PAD_A // 16], i16)
    widxA = inp("widxA", [128, NTA])
    sclA = inp("sclA", [128, NTA])
    idxB = inp("idxB", [128, E_PAD_B // 16], i16)
    widxB = inp("widxB", [128, NTB])
    sclB = inp("sclB", [128, NTB])
    pool0_r = inp("pool0_r", [128, NK_PAD], bf16)
    pool1_r = inp("pool1_r", [128, NK_PAD], bf16)
    W0_d = inp("W0", [F_IN, H], bf16)
    Wc1_d = inp("Wc1", [H, H], bf16)
    Wc2_d = inp("Wc2", [H, H], bf16)
    WgA_d = inp("WgA", [H, H])
    WgB_d = inp("WgB", [H, H])
    W1_d = inp("W1f", [H, 64])
    W2_d = inp("W2f", [64, 32])
    W3_d = inp("W3", [32, 4])
    b0_d = inp("b0c", [H, 1])
    bc1_d = inp("bc1c", [H, 1])
    bc2_d = inp("bc2c", [H, 1])
    bg_d = inp("bgc", [H, 1])
    b1_d = inp("b1c", [64, 1])
    b2_d = inp("b2c", [32, 1])
    out_d = nc.dram_tensor("out", [4, 2], f32, kind="ExternalOutput")

    dbg = {"kind": "ExternalOutput"} if KDEBUG else {}
    z1_h = nc.dram_tensor("z1_h", [NK_PAD, H], bf16, **dbg)
    z2_h = nc.dram_tensor("z2_h", [NK_PAD, H], bf16, **dbg)
    eA1 = nc.dram_tensor("eA1", [M_PAD, H], bf16)
    eR1 = nc.dram_tensor("eR1", [M_PAD, H], bf16, addr_space="Shared")
    eA2 = nc.dram_tensor("eA2", [M_PAD, H], bf16)
    eR2 = nc.dram_tensor("eR2", [M_PAD, H], bf16, addr_space="Shared")
    if KDEBUG:
        dumpR1 = nc.dram_tensor("dumpR1", [M_PAD, H], bf16, kind="ExternalOutput")
        dumpR2 = nc.dram_tensor("dumpR2", [M_PAD, H], bf16, kind="ExternalOutput")

    with TileContext(nc) as tc:
        with (
            tc.tile_pool(name="c", bufs=1) as cpool,
            tc.tile_pool(name="g", bufs=2) as gpool,
            tc.tile_pool(name="oh", bufs=6) as ohpool,
            tc.tile_pool(name="s", bufs=3) as spool,
            tc.tile_pool(name="ps", bufs=2, space="PSUM") as pspool,
            tc.tile_pool(name="pm", bufs=1, space="PSUM") as pmpool,
            tc.tile_pool(name="acc", bufs=1) as accpool,
            tc.tile_pool(name="bc", bufs=2) as bcpool,
        ):
            nc.gpsimd.load_library(library_config.mlp)
            dma_sem = nc.alloc_semaphore("swdge_dma")
            GATHER_KW = (
                {"prepare_only": True, "sem": dma_sem}
                if PREP_GATHER else {})

            # ---- constants ----
            iota_i = cpool.tile([128, WB], i32)
            nc.gpsimd.iota(iota_i[:], [[1, WB]], channel_multiplier=0)
            iota_bf = cpool.tile([128, WB], bf16)
            nc.vector.tensor_copy(iota_bf[:], iota_i[:])
            idn_i = cpool.tile([128, 128], i32)
            nc.gpsimd.iota(idn_i[:], [[1, 128]], channel_multiplier=-1)
            ident = cpool.tile([128, 128], bf16)
            nc.vector.tensor_scalar(ident[:], idn_i[:], 0.0, None, EQ)

            def load_sb(dram, shape, dt=f32):
                t = cpool.tile(shape, dt, tag=dram.name + "_sb")
                nc.sync.dma_start(out=t[:], in_=dram[:, :])
                return t

            W0s = load_sb(W0_d, [F_IN, H], bf16)
            Wc1s = load_sb(Wc1_d, [H, H], bf16)
            Wc2s = load_sb(Wc2_d, [H, H], bf16)
            WgAs = load_sb(WgA_d, [H, H])
            WgBs = load_sb(WgB_d, [H, H])
            W1s = load_sb(W1_d, [H, 64])
            W2s = load_sb(W2_d, [64, 32])
            W3s = load_sb(W3_d, [32, 4])
            b0s = load_sb(b0_d, [H, 1])
            bc1s = load_sb(bc1_d, [H, 1])
            bc2s = load_sb(bc2_d, [H, 1])
            bgs = load_sb(bg_d, [H, 1])
            b1s = load_sb(b1_d, [64, 1])
            b2s = load_sb(b2_d, [32, 1])
            idxAs = load_sb(idxA, [128, E_PAD_A // 16], i16)
            widxAs = load_sb(widxA, [128, NTA])
            sclAs = load_sb(sclA, [128, NTA])
            idxBs = load_sb(idxB, [128, E_PAD_B // 16], i16)
            widxBs = load_sb(widxB, [128, NTB])
            sclBs = load_sb(sclB, [128, NTB])

            p1acc = accpool.tile([128, 2], f32)
            nc.vector.memset(p1acc[:], 0.0)
            p2acc = accpool.tile([128, 2], f32)
            nc.vector.memset(p2acc[:], 0.0)

            def wr_z(zc, dst, row0):
                """Transpose [128f, 256n] bf16 chunk to node rows, DMA out."""
                for h_ in range(2):
                    tp = pspool.tile([128, 128], bf16, tag="trp")
                    nc.tensor.transpose(
                        tp[:], zc[:, h_ * 128:(h_ + 1) * 128], ident[:])
                    rows = spool.tile([128, 128], bf16, tag="rows")
                    nc.scalar.copy(rows[:], tp[:])
                    nc.sync.dma_start(
                        out=dst[row0 + h_ * 128:row0 + h_ * 128 + 128, :],
                        in_=rows[:])

            # ---- stage 1+2: z1 = (relu(x@W0+b0)) @ Wc1, node rows to HBM ----
            for c in range(NWB):
                xc = spool.tile([F_IN, WB], bf16, tag="xc")
                nc.sync.dma_start(out=xc[:], in_=xT[:, c * WB:(c + 1) * WB])
                ps1 = pspool.tile([128, WB], f32, tag="st")
                nc.tensor.matmul(ps1[:], W0s[:], xc[:], start=True, stop=True)
                h1c = spool.tile([128, WB], bf16, tag="h1c")
                nc.scalar.activation(h1c[:], ps1[:], RELU, bias=b0s[:, 0:1])
                ps2 = pspool.tile([128, WB], f32, tag="st")
                nc.tensor.matmul(ps2[:], Wc1s[:], h1c[:], start=True, stop=True)
                zc = spool.tile([128, WB], bf16, tag="zc")
                nc.scalar.copy(zc[:], ps2[:])
                wr_z(zc, z1_h, c * WB)

            # ---- conv phase A: partial hyperedge sums (hedge-major PSUM) ----
            def phase_A(table_h, out_h, red_h):
                for ch in range(NCH_A):
                    g = gpool.tile([128, CHW_A * TW_A, 128], bf16, tag="gA")
                    nc.gpsimd.dma_gather(
                        g[:], table_h.ap(),
                        idxAs[:, ch * (NI_A // 16):(ch + 1) * (NI_A // 16)],
                        NI_A, NI_A, 128, single_packet=False,
                        **GATHER_KW,
                    )
                    if GATHER_KW.get("prepare_only"):
                        nc.gpsimd.trigger_dma(count=None)
                    for wi in range(CHW_A):
                        w = ch * CHW_A + wi
                        psw = pspool.tile([128, WB], f32, tag="ph")
                        ps = psw[:, :WA]
                        for tt in range(TW_A):
                            tg = w * TW_A + tt
                            oh = ohpool.tile([128, 128], bf16, tag="ohA")
                            nc.vector.tensor_scalar(
                                oh[:], iota_bf[:, :WA],
                                widxAs[:, tg:tg + 1], sclAs[:, tg:tg + 1],
                                EQ, MULT)
                            nc.tensor.matmul(
                                ps[:], oh[:], g[:, wi * TW_A + tt, :],
                                start=(tt == 0), stop=(tt == TW_A - 1))
                        rows = spool.tile([128, 128], bf16, tag="rowsA")
                        nc.scalar.copy(rows[:], ps[:])
                        nc.sync.dma_start(
                            out=out_h[w * WA:(w + 1) * WA, :], in_=rows[:])
                    if ch == AR_SPLIT_W // CHW_A - 1:
                        nc.gpsimd.collective_compute(
                            "AllReduce", ADD,
                            replica_groups=[list(range(NCORE))],
                            ins=[out_h[0:AR_SPLIT, :].opt()],
                            outs=[red_h[0:AR_SPLIT, :].opt()])
                nc.gpsimd.collective_compute(
                    "AllReduce", ADD, replica_groups=[list(range(NCORE))],
                    ins=[out_h[AR_SPLIT:M_PAD, :].opt()],
                    outs=[red_h[AR_SPLIT:M_PAD, :].opt()])

            # ---- conv phase B: node sums + relu + pooling (+ next z) ----
            def phase_B(table_h, bias_s, pacc, Wnext, znext_h):
                for ch in range(NCH_B):
                    g = gpool.tile([128, CHW_B * TW_B, 128], bf16, tag="gB")
                    nc.gpsimd.dma_gather(
                        g[:], table_h.ap(),
                        idxBs[:, ch * (NI_B // 16):(ch + 1) * (NI_B // 16)],
                        NI_B, NI_B, 128, single_packet=False,
                        **GATHER_KW,
                    )
                    if GATHER_KW.get("prepare_only"):
                        nc.gpsimd.trigger_dma(count=None)
                    csl = slice(ch * CHW_B * WB, (ch + 1) * CHW_B * WB)
                    p0ch = bcpool.tile([128, CHW_B * WB], bf16, tag="p0ch")
                    nc.sync.dma_start(out=p0ch[:], in_=pool0_r[:, csl])
                    p1ch = bcpool.tile([128, CHW_B * WB], bf16, tag="p1ch")
                    nc.sync.dma_start(out=p1ch[:], in_=pool1_r[:, csl])
                    for wi in range(CHW_B):
                        w = ch * CHW_B + wi
                        psw = pspool.tile([128, WB], f32, tag="ph")
                        ps = psw[:]
                        for tt in range(TW_B):
                            tg = w * TW_B + tt
                            oh = ohpool.tile([128, WB], bf16, tag="ohB")
                            eng = nc.gpsimd if tt % 3 == 2 else nc.vector
                            eng.tensor_scalar(
                                oh[:], iota_bf[:],
                                widxBs[:, tg:tg + 1], sclBs[:, tg:tg + 1],
                                EQ, MULT)
                            nc.tensor.matmul(
                                ps[:], g[:, wi * TW_B + tt, :], oh[:],
                                start=(tt == 0), stop=(tt == TW_B - 1))
                        h2c = spool.tile([128, WB], bf16, tag="h2c")
                        nc.scalar.activation(
                            h2c[:], ps[:], RELU, bias=bias_s[:, 0:1])
                        sl = slice(wi * WB, (wi + 1) * WB)
                        for gi, prow in ((0, p0ch), (1, p1ch)):
                            pm = spool.tile([128, WB], bf16, tag="pm")
                            nc.vector.tensor_mul(pm[:], h2c[:], prow[:, sl])
                            rs = spool.tile([128, 1], f32, tag="rs")
                            nc.vector.tensor_reduce(
                                rs[:], pm[:], mybir.AxisListType.X, ADD)
                            nc.vector.tensor_add(
                                pacc[:, gi:gi + 1], pacc[:, gi:gi + 1], rs[:])
                        if znext_h is not None:
                            zp = pspool.tile([128, WB], f32, tag="st")
                            nc.tensor.matmul(
                                zp[:], Wnext[:], h2c[:], start=True, stop=True)
                            zc = spool.tile([128, WB], bf16, tag="zc")
                            nc.scalar.copy(zc[:], zp[:])
                            wr_z(zc, znext_h, w * WB)

            def dump_table(src_h, dst_h):
                for w in range(NWA):
                    t = spool.tile([128, H], bf16, tag="dmp")
                    nc.sync.dma_start(
                        out=t[:], in_=src_h[w * WA:(w + 1) * WA, :])
                    nc.sync.dma_start(
                        out=dst_h[w * WA:(w + 1) * WA, :], in_=t[:])

            phase_A(z1_h, eA1, eR1)
            if KDEBUG:
                dump_table(eR1, dumpR1)
            phase_B(eR1, bc1s, p1acc, Wc2s, z2_h)
            phase_A(z2_h, eA2, eR2)
            if KDEBUG:
                dump_table(eR2, dumpR2)
            phase_B(eR2, bc2s, p2acc, None, None)

            # ---- MLP head on the two pooled rows (fp32) ----
            gpsw = pmpool.tile([128, 2], f32, tag="mlp")
            gps = gpsw[:]
            nc.tensor.matmul(gps, WgAs[:], p1acc[:], start=True, stop=False)
            nc.tensor.matmul(gps, WgBs[:], p2acc[:], start=False, stop=True)
            gb = spool.tile([128, 2], f32, tag="m1")
            nc.vector.tensor_scalar(gb[:], gps, bgs[:, 0:1], None, ADD)
            h1psw = pmpool.tile([128, 2], f32, tag="mlp")
            h1ps = h1psw[:64, :]
            nc.tensor.matmul(h1ps, W1s[:], gb[:], start=True, stop=True)
            h1m = spool.tile([64, 2], f32, tag="m2")
            nc.vector.tensor_scalar(
                h1m[:], h1ps, b1s[:, 0:1], 0.0, ADD, mybir.AluOpType.max)
            h2psw = pmpool.tile([128, 2], f32, tag="mlp")
            h2ps = h2psw[:32, :]
            nc.tensor.matmul(h2ps, W2s[:], h1m[:], start=True, stop=True)
            h2m = spool.tile([32, 2], f32, tag="m3")
            nc.vector.tensor_scalar(
                h2m[:], h2ps, b2s[:, 0:1], 0.0, ADD, mybir.AluOpType.max)
            opsw = pmpool.tile([128, 2], f32, tag="mlp")
            ops = opsw[:4, :]
            nc.tensor.matmul(ops, W3s[:], h2m[:], start=True, stop=True)
            om = spool.tile([4, 2], f32, tag="m4")
            nc.vector.tensor_copy(om[:], ops)
            nc.sync.dma_start(out=out_d[:, :], in_=om[:])

    nc.compile()
    return nc


def _wrap_idx(idx):
    return np.tile(idx.reshape(-1, 16).T, (8, 1)).copy()


def _prep_core(k, x, node_idx, hedge_idx, batch, binv, bf16):
    s = int(np.searchsorted(batch, 2 * k))
    e = int(np.searchsorted(batch, 2 * k + 2))
    nk = e - s
    assert nk <= NK_PAD
    sel = np.where((node_idx >= s) & (node_idx < e))[0]
    na = (node_idx[sel] - s).astype(np.int64)
    ha = hedge_idx[sel].astype(np.int64)

    deg = np.bincount(node_idx, minlength=N).astype(np.float32)
    dl = deg[s:e]
    dinv = np.zeros(NK_PAD, np.float32)
    dinv[:nk] = np.where(dl > 0, 1.0 / np.maximum(dl, 1), 0.0)

    def build(keys, vals, scale, nwin, tw, width):
        order = np.argsort(keys, kind="stable")
        ks, vs, sc = keys[order], vals[order], scale[order]
        gidx = np.zeros(nwin * tw * 128, np.int16)
        widx = np.full(nwin * tw * 128, -1.0, np.float32)
        sedg = np.zeros(nwin * tw * 128, np.float32)
        starts = np.searchsorted(ks, np.arange(nwin) * width)
        ends = np.searchsorted(ks, (np.arange(nwin) + 1) * width)
        for w in range(nwin):
            a, b = starts[w], ends[w]
            n = b - a
            assert n <= tw * 128, f"window overflow {n} > {tw * 128}"
            o = w * tw * 128
            gidx[o:o + n] = vs[a:b]
            widx[o:o + n] = ks[a:b] - w * width
            sedg[o:o + n] = sc[a:b]
        return (_wrap_idx(gidx),
                widx.reshape(-1, 128).T.copy(),
                sedg.reshape(-1, 128).T.copy())

    # phase A: edges sorted by hedge; gather z rows (na); scale = binv[ha]
    giA, wiA, scA = build(ha, na, binv[ha], NWA, TW_A, WA)
    # phase B: edges sorted by node; gather edge rows (ha); scale = dinv[na]
    giB, wiB, scB = build(na, ha, dinv[na], NWB, TW_B, WB)

    xT = np.zeros((F_IN, NK_PAD), bf16)
    xT[:, :nk] = x[s:e].T.astype(bf16)

    p0 = np.zeros(NK_PAD, np.float32)
    p1 = np.zeros(NK_PAD, np.float32)
    bloc = batch[s:e]
    for gi, arr in ((2 * k, p0), (2 * k + 1, p1)):
        m = (bloc == gi)
        cnt = max(float(m.sum()), 1.0)
        arr[:nk][m] = 1.0 / cnt
    return {
        "xT": xT,
        "idxA": giA, "widxA": wiA, "sclA": scA,
        "idxB": giB, "widxB": wiB, "sclB": scB,
        "pool0_r": np.ascontiguousarray(
            np.broadcast_to(p0.astype(bf16), (128, NK_PAD))),
        "pool1_r": np.ascontiguousarray(
            np.broadcast_to(p1.astype(bf16), (128, NK_PAD))),
    }


def build_in_maps(x, node_idx, hedge_idx, batch, W0, b0, Wc1, bc1, Wc2, bc2,
                  Wg, bg, W1, b1, g1, be1, rm1, rv1, W2, b2, g2, be2, rm2,
                  rv2, W3):
    import ml_dtypes

    bf16 = ml_dtypes.bfloat16

    x = np.asarray(x, np.float32)
    node_idx = np.asarray(node_idx).astype(np.int64)
    hedge_idx = np.asarray(hedge_idx).astype(np.int64)
    batch_np = np.asarray(batch).astype(np.int64)

    # eval-BN folded into W1/W2
    k1 = np.asarray(g1) / np.sqrt(np.asarray(rv1) + EPS)
    W1f = (np.asarray(W1) * k1[None, :]).astype(np.float32)
    b1f = ((np.asarray(b1) - np.asarray(rm1)) * k1 + np.asarray(be1)).astype(
        np.float32)
    k2 = np.asarray(g2) / np.sqrt(np.asarray(rv2) + EPS)
    W2f = (np.asarray(W2) * k2[None, :]).astype(np.float32)
    b2f = ((np.asarray(b2) - np.asarray(rm2)) * k2 + np.asarray(be2)).astype(
        np.float32)

    cnt = np.bincount(hedge_idx, minlength=M_PAD).astype(np.float32)
    binv = np.where(cnt > 0, 1.0 / np.maximum(cnt, 1), 0.0).astype(np.float32)

    Wg_np = np.asarray(Wg, np.float32)
    shared = {
        "W0": np.asarray(W0, np.float32).astype(bf16),
        "Wc1": np.asarray(Wc1, np.float32).astype(bf16),
        "Wc2": np.asarray(Wc2, np.float32).astype(bf16),
        "WgA": Wg_np[:H], "WgB": Wg_np[H:],
        "W1f": W1f, "W2f": W2f, "W3": np.asarray(W3, np.float32),
        "b0c": np.asarray(b0, np.float32).reshape(-1, 1),
        "bc1c": np.asarray(bc1, np.float32).reshape(-1, 1),
        "bc2c": np.asarray(bc2, np.float32).reshape(-1, 1),
        "bgc": np.asarray(bg, np.float32).reshape(-1, 1),
        "b1c": b1f.reshape(-1, 1), "b2c": b2f.reshape(-1, 1),
    }
    in_maps = []
    for k in range(NCORE):
        m = _prep_core(k, x, node_idx, hedge_idx, batch_np, binv, bf16)
        m.update(shared)
        in_maps.append(m)
    return in_maps


def kernel(x, node_idx, hedge_idx, batch, W0, b0, Wc1, bc1, Wc2, bc2,
           Wg, bg, W1, b1, g1, be1, rm1, rv1, W2, b2, g2, be2, rm2, rv2, W3):
    global _COMPILED
    from concourse.bass_utils import run_bass_kernel_spmd

    if _COMPILED is None:
        _COMPILED = _build_nc()
    nc = _COMPILED

    in_maps = build_in_maps(
        x, node_idx, hedge_idx, batch, W0, b0, Wc1, bc1, Wc2, bc2, Wg, bg,
        W1, b1, g1, be1, rm1, rv1, W2, b2, g2, be2, rm2, rv2, W3)

    r = run_bass_kernel_spmd(nc, in_maps, core_ids=list(range(NCORE)))
    out = np.zeros((B_GRAPHS, 4), np.float32)
    for k in range(NCORE):
        o = r.results[k]["out"]
        out[2 * k] = o[:, 0]
        out[2 * k + 1] = o[:, 1]
    return out


# revision 14
# speedup vs baseline: 1.1020x; 1.1020x over previous
"""Trainium2 Bass kernel for the hypergraph-conv survival model.

Sharding: graph/data parallel over 8 NeuronCores. Core k owns graphs
2k,2k+1 (a contiguous node range, since `batch` is sorted). Both hconv
message-passing directions are sharded by that node range:
  phase A: gather local z rows by incidence sorted by hyperedge
           (SWDGE prepare_only gathers round-robined over 4 queues so
           DMA drains overlap); segment-sum via bf16 matmuls against
           host-precomputed one-hot tiles streamed from HBM. The
           one-hot is the stationary operand, so PSUM comes out
           hyperedge-major and needs no transpose; B^-1 is folded into
           the one-hot values per edge.
  AllReduce the [25088,128] bf16 partial hyperedge table (in 2 chunks,
           the first overlapping the tail of phase A).
  phase B: gather reduced hyperedge rows by incidence sorted by node;
           segment-sum into 128-wide node windows feature-major with
           D^-1 folded into the streamed one-hot; bias+relu on the Act
           engine; mean-pool weights applied on DVE.
Weights are replicated; the final MLP runs per-core on its two pooled
graph rows, all in fp32.
"""

import os
import sys

sys.path.insert(0, "/opt/trn_rl_repo")

import numpy as np

KDEBUG = bool(int(os.environ.get("KDEBUG", "0")))
NQ = int(os.environ.get("KNQ", "4"))        # SWDGE queues for gathers

# ---- static problem sizes (from the reference) ----
N = 100_000
E = 800_000
M = 25_000
B_GRAPHS = 16
F_IN = 64
H = 128
EPS = 1e-5
NCORE = 8

NK_PAD = 13312          # padded per-core node count (104*128)
M_PAD = 25088           # padded hyperedge count  (196*128)

WA = 128                # phase A window width (hyperedges)
NWA = M_PAD // WA       # 196 hedge windows
TW_A = 5                # tiles (128 edges) per hedge window (max seen 593)
NTA = NWA * TW_A        # 980 tiles
E_PAD_A = NTA * 128     # 125440
CHW_A = 14              # windows per gather chunk -> 14 chunks
NCH_A = NWA // CHW_A
NI_A = CHW_A * TW_A * 128   # idxs per phase-A gather (8960)

WB = 128                # phase B window width (nodes)
NWB = NK_PAD // WB      # 104 node windows
TW_B = 10               # tiles per node window (max seen 1165)
NTB = NWB * TW_B        # 1040 tiles
E_PAD_B = NTB * 128     # 133120
CHW_B = 8               # windows per gather chunk -> 13 chunks
NCH_B = NWB // CHW_B
NI_B = CHW_B * TW_B * 128   # idxs per phase-B gather (10240)

GT = 80                 # unified gather/one-hot chunk tile count (>=70,80)

AR_SPLIT_W = 98         # phase-A windows in the first AllReduce chunk
AR_SPLIT = AR_SPLIT_W * WA  # 12544 rows

_COMPILED = None


def _build_nc():
    import concourse.bacc as bacc
    import concourse.mybir as mybir
    from concourse.tile import TileContext
    from concourse import library_config

    f32 = mybir.dt.float32
    bf16 = mybir.dt.bfloat16
    i16 = mybir.dt.int16
    i32 = mybir.dt.int32
    EQ = mybir.AluOpType.is_equal
    ADD = mybir.AluOpType.add
    RELU = mybir.ActivationFunctionType.Relu

    nc = bacc.Bacc("TRN2", target_bir_lowering=False, num_devices=NCORE,
                   num_swdge_queues=NQ)

    def inp(name, shape, dt=f32):
        return nc.dram_tensor(name, shape, dt, kind="ExternalInput")

    xT = inp("xT", [F_IN, NK_PAD], bf16)
    idxA = inp("idxA", [128, E_PAD_A // 16], i16)
    idxB = inp("idxB", [128, E_PAD_B // 16], i16)
    ohA_d = inp("ohA", [128, NTA * 128], bf16)
    ohB_d = inp("ohB", [128, NTB * 128], bf16)
    pool0_r = inp("pool0_r", [128, NK_PAD], bf16)
    pool1_r = inp("pool1_r", [128, NK_PAD], bf16)
    W0_d = inp("W0", [F_IN, H], bf16)
    Wc1_d = inp("Wc1", [H, H], bf16)
    Wc2_d = inp("Wc2", [H, H], bf16)
    WgA_d = inp("WgA", [H, H])
    WgB_d = inp("WgB", [H, H])
    W1_d = inp("W1f", [H, 64])
    W2_d = inp("W2f", [64, 32])
    W3_d = inp("W3", [32, 4])
    b0_d = inp("b0c", [H, 1])
    bc1_d = inp("bc1c", [H, 1])
    bc2_d = inp("bc2c", [H, 1])
    bg_d = inp("bgc", [H, 1])
    b1_d = inp("b1c", [64, 1])
    b2_d = inp("b2c", [32, 1])
    out_d = nc.dram_tensor("out", [4, 2], f32, kind="ExternalOutput")

    dbg = {"kind": "ExternalOutput"} if KDEBUG else {}
    z1_h = nc.dram_tensor("z1_h", [NK_PAD, H], bf16, **dbg)
    z2_h = nc.dram_tensor("z2_h", [NK_PAD, H], bf16, **dbg)
    eA1 = nc.dram_tensor("eA1", [M_PAD, H], bf16)
    eR1 = nc.dram_tensor("eR1", [M_PAD, H], bf16, addr_space="Shared")
    eA2 = nc.dram_tensor("eA2", [M_PAD, H], bf16)
    eR2 = nc.dram_tensor("eR2", [M_PAD, H], bf16, addr_space="Shared")
    if KDEBUG:
        dumpR1 = nc.dram_tensor("dumpR1", [M_PAD, H], bf16, kind="ExternalOutput")
        dumpR2 = nc.dram_tensor("dumpR2", [M_PAD, H], bf16, kind="ExternalOutput")

    with TileContext(nc) as tc:
        with (
            tc.tile_pool(name="c", bufs=1) as cpool,
            tc.tile_pool(name="g", bufs=4) as gpool,
            tc.tile_pool(name="oh", bufs=2) as ohpool,
            tc.tile_pool(name="s", bufs=3) as spool,
            tc.tile_pool(name="ps", bufs=2, space="PSUM") as pspool,
            tc.tile_pool(name="pm", bufs=1, space="PSUM") as pmpool,
            tc.tile_pool(name="acc", bufs=1) as accpool,
            tc.tile_pool(name="bc", bufs=2) as bcpool,
        ):
            nc.gpsimd.load_library(library_config.mlp)
            sems = [nc.alloc_semaphore(f"swdge_dma{q}") for q in range(NQ)]
            uses = [0] * NQ
            gctr = [0]

            # ---- constants ----
            idn_i = cpool.tile([128, 128], i32)
            nc.gpsimd.iota(idn_i[:], [[1, 128]], channel_multiplier=-1)
            ident = cpool.tile([128, 128], bf16)
            nc.vector.tensor_scalar(ident[:], idn_i[:], 0.0, None, EQ)

            def load_sb(dram, shape, dt=f32):
                t = cpool.tile(shape, dt, tag=dram.name + "_sb")
                nc.sync.dma_start(out=t[:], in_=dram[:, :])
                return t

            W0s = load_sb(W0_d, [F_IN, H], bf16)
            Wc1s = load_sb(Wc1_d, [H, H], bf16)
            Wc2s = load_sb(Wc2_d, [H, H], bf16)
            WgAs = load_sb(WgA_d, [H, H])
            WgBs = load_sb(WgB_d, [H, H])
            W1s = load_sb(W1_d, [H, 64])
            W2s = load_sb(W2_d, [64, 32])
            W3s = load_sb(W3_d, [32, 4])
            b0s = load_sb(b0_d, [H, 1])
            bc1s = load_sb(bc1_d, [H, 1])
            bc2s = load_sb(bc2_d, [H, 1])
            bgs = load_sb(bg_d, [H, 1])
            b1s = load_sb(b1_d, [64, 1])
            b2s = load_sb(b2_d, [32, 1])
            idxAs = load_sb(idxA, [128, E_PAD_A // 16], i16)
            idxBs = load_sb(idxB, [128, E_PAD_B // 16], i16)

            p1acc = accpool.tile([128, 2], f32)
            nc.vector.memset(p1acc[:], 0.0)
            p2acc = accpool.tile([128, 2], f32)
            nc.vector.memset(p2acc[:], 0.0)

            def gather(table_h, idx_sb, i0, ni):
                """prepare_only gather of ni rows; returns (tile, wait)."""
                g = gpool.tile([128, GT, 128], bf16, tag="g")
                q = gctr[0] % NQ
                gctr[0] += 1
                nc.gpsimd.dma_gather(
                    g[:, :ni // 128, :], table_h.ap(),
                    idx_sb[:, i0 // 16:(i0 + ni) // 16],
                    ni, ni, 128, single_packet=False,
                    prepare_only=True, sem=sems[q], queue_num=q,
                )
                nc.gpsimd.trigger_dma(count=None, queue_num=q)
                uses[q] += 1
                target = 16 * uses[q]
                sem = sems[q]

                def wait(eng):
                    eng.wait_ge(sem, target)
                return g, wait

            def wr_rows(zc, dst, row0):
                """Transpose [128f, 128n] bf16 block to node rows, DMA out."""
                tp = pspool.tile([128, 128], bf16, tag="trp")
                nc.tensor.transpose(tp[:], zc[:], ident[:])
                rows = spool.tile([128, 128], bf16, tag="rows")
                nc.scalar.copy(rows[:], tp[:])
                nc.sync.dma_start(out=dst[row0:row0 + 128, :], in_=rows[:])

            # ---- stage 1+2: z1 = (relu(x@W0+b0)) @ Wc1, node rows to HBM ----
            for c in range(NWB):
                xc = spool.tile([F_IN, WB], bf16, tag="xc")
                nc.sync.dma_start(out=xc[:], in_=xT[:, c * WB:(c + 1) * WB])
                ps1 = pspool.tile([128, WB], f32, tag="st")
                nc.tensor.matmul(ps1[:], W0s[:], xc[:], start=True, stop=True)
                h1c = spool.tile([128, WB], bf16, tag="h1c")
                nc.scalar.activation(h1c[:], ps1[:], RELU, bias=b0s[:, 0:1])
                ps2 = pspool.tile([128, WB], f32, tag="st")
                nc.tensor.matmul(ps2[:], Wc1s[:], h1c[:], start=True, stop=True)
                zc = spool.tile([128, WB], bf16, tag="zc")
                nc.scalar.copy(zc[:], ps2[:])
                wr_rows(zc, z1_h, c * WB)

            # ---- conv phase A: partial hyperedge sums (hedge-major PSUM) ----
            def phase_A(table_h, out_h, red_h):
                for ch in range(NCH_A):
                    g, gwait = gather(table_h, idxAs, ch * NI_A, NI_A)
                    ohs = ohpool.tile([128, GT, 128], bf16, tag="ohs")
                    nc.sync.dma_start(
                        out=ohs[:, :CHW_A * TW_A, :],
                        in_=ohA_d[:, ch * NI_A:(ch + 1) * NI_A])
                    gwait(nc.tensor)
                    for wi in range(CHW_A):
                        w = ch * CHW_A + wi
                        psw = pspool.tile([128, WB], f32, tag="ph")
                        ps = psw[:]
                        for tt in range(TW_A):
                            t = wi * TW_A + tt
                            nc.tensor.matmul(
                                ps, ohs[:, t, :], g[:, t, :],
                                start=(tt == 0), stop=(tt == TW_A - 1))
                        rows = spool.tile([128, 128], bf16, tag="rowsA")
                        nc.scalar.copy(rows[:], ps)
                        nc.sync.dma_start(
                            out=out_h[w * WA:(w + 1) * WA, :], in_=rows[:])
                    if ch == AR_SPLIT_W // CHW_A - 1:
                        nc.gpsimd.collective_compute(
                            "AllReduce", ADD,
                            replica_groups=[list(range(NCORE))],
                            ins=[out_h[0:AR_SPLIT, :].opt()],
                            outs=[red_h[0:AR_SPLIT, :].opt()])
                nc.gpsimd.collective_compute(
                    "AllReduce", ADD, replica_groups=[list(range(NCORE))],
                    ins=[out_h[AR_SPLIT:M_PAD, :].opt()],
                    outs=[red_h[AR_SPLIT:M_PAD, :].opt()])

            # ---- conv phase B: node sums + relu + pooling (+ next z) ----
            def phase_B(table_h, bias_s, pacc, Wnext, znext_h):
                for ch in range(NCH_B):
                    g, gwait = gather(table_h, idxBs, ch * NI_B, NI_B)
                    ohs = ohpool.tile([128, GT, 128], bf16, tag="ohs")
                    nc.sync.dma_start(
                        out=ohs[:, :CHW_B * TW_B, :],
                        in_=ohB_d[:, ch * NI_B:(ch + 1) * NI_B])
                    csl = slice(ch * CHW_B * WB, (ch + 1) * CHW_B * WB)
                    p0ch = bcpool.tile([128, CHW_B * WB], bf16, tag="p0ch")
                    nc.sync.dma_start(out=p0ch[:], in_=pool0_r[:, csl])
                    p1ch = bcpool.tile([128, CHW_B * WB], bf16, tag="p1ch")
                    nc.sync.dma_start(out=p1ch[:], in_=pool1_r[:, csl])
                    gwait(nc.tensor)
                    for wi in range(CHW_B):
                        w = ch * CHW_B + wi
                        psw = pspool.tile([128, WB], f32, tag="ph")
                        ps = psw[:]
                        for tt in range(TW_B):
                            t = wi * TW_B + tt
                            nc.tensor.matmul(
                                ps, g[:, t, :], ohs[:, t, :],
                                start=(tt == 0), stop=(tt == TW_B - 1))
                        h2c = spool.tile([128, WB], bf16, tag="h2c")
                        nc.scalar.activation(
                            h2c[:], ps, RELU, bias=bias_s[:, 0:1])
                        sl = slice(wi * WB, (wi + 1) * WB)
                        for gi, prow in ((0, p0ch), (1, p1ch)):
                            pm = spool.tile([128, WB], bf16, tag="pm")
                            nc.vector.tensor_mul(pm[:], h2c[:], prow[:, sl])
                            rs = spool.tile([128, 1], f32, tag="rs")
                            nc.vector.tensor_reduce(
                                rs[:], pm[:], mybir.AxisListType.X, ADD)
                            nc.vector.tensor_add(
                                pacc[:, gi:gi + 1], pacc[:, gi:gi + 1], rs[:])
                        if znext_h is not None:
                            zp = pspool.tile([128, WB], f32, tag="st")
                            nc.tensor.matmul(
                                zp[:], Wnext[:], h2c[:], start=True, stop=True)
                            zc = spool.tile([128, WB], bf16, tag="zc")
                            nc.scalar.copy(zc[:], zp[:])
                            wr_rows(zc, znext_h, w * WB)

            def dump_table(src_h, dst_h):
                for w in range(NWA):
                    t = spool.tile([128, H], bf16, tag="dmp")
                    nc.sync.dma_start(
                        out=t[:], in_=src_h[w * WA:(w + 1) * WA, :])
                    nc.sync.dma_start(
                        out=dst_h[w * WA:(w + 1) * WA, :], in_=t[:])

            phase_A(z1_h, eA1, eR1)
            if KDEBUG:
                dump_table(eR1, dumpR1)
            phase_B(eR1, bc1s, p1acc, Wc2s, z2_h)
            phase_A(z2_h, eA2, eR2)
            if KDEBUG:
                dump_table(eR2, dumpR2)
            phase_B(eR2, bc2s, p2acc, None, None)

            # ---- MLP head on the two pooled rows (fp32) ----
            gpsw = pmpool.tile([128, 2], f32, tag="mlp")
            gps = gpsw[:]
            nc.tensor.matmul(gps, WgAs[:], p1acc[:], start=True, stop=False)
            nc.tensor.matmul(gps, WgBs[:], p2acc[:], start=False, stop=True)
            gb = spool.tile([128, 2], f32, tag="m1")
            nc.vector.tensor_scalar(gb[:], gps, bgs[:, 0:1], None, ADD)
            h1psw = pmpool.tile([128, 2], f32, tag="mlp")
            h1ps = h1psw[:64, :]
            nc.tensor.matmul(h1ps, W1s[:], gb[:], start=True, stop=True)
            h1m = spool.tile([64, 2], f32, tag="m2")
            nc.vector.tensor_scalar(
                h1m[:], h1ps, b1s[:, 0:1], 0.0, ADD, mybir.AluOpType.max)
            h2psw = pmpool.tile([128, 2], f32, tag="mlp")
            h2ps = h2psw[:32, :]
            nc.tensor.matmul(h2ps, W2s[:], h1m[:], start=True, stop=True)
            h2m = spool.tile([32, 2], f32, tag="m3")
            nc.vector.tensor_scalar(
                h2m[:], h2ps, b2s[:, 0:1], 0.0, ADD, mybir.AluOpType.max)
            opsw = pmpool.tile([128, 2], f32, tag="mlp")
            ops = opsw[:4, :]
            nc.tensor.matmul(ops, W3s[:], h2m[:], start=True, stop=True)
            om = spool.tile([4, 2], f32, tag="m4")
            nc.vector.tensor_copy(om[:], ops)
            nc.sync.dma_start(out=out_d[:, :], in_=om[:])

    nc.compile()
    return nc


def _wrap_idx(idx):
    return np.tile(idx.reshape(-1, 16).T, (8, 1)).copy()


def _make_oh(widx, scale, ntiles, bf16):
    """[128, ntiles*128] bf16 one-hot tiles: partition=edge-in-tile,
    col block t holds tile t; oh[e, c] = scale[e] if widx[e]==c else 0."""
    flat = np.zeros((ntiles * 128, 128), np.float32)
    valid = widx >= 0
    rows = np.nonzero(valid)[0]
    flat[rows, widx[valid]] = scale[valid]
    return np.ascontiguousarray(
        flat.reshape(ntiles, 128, 128).transpose(1, 0, 2)
        .reshape(128, ntiles * 128)).astype(bf16)


def _prep_core(k, x, node_idx, hedge_idx, batch, binv, bf16):
    s = int(np.searchsorted(batch, 2 * k))
    e = int(np.searchsorted(batch, 2 * k + 2))
    nk = e - s
    assert nk <= NK_PAD
    sel = np.where((node_idx >= s) & (node_idx < e))[0]
    na = (node_idx[sel] - s).astype(np.int64)
    ha = hedge_idx[sel].astype(np.int64)

    deg = np.bincount(node_idx, minlength=N).astype(np.float32)
    dl = deg[s:e]
    dinv = np.zeros(NK_PAD, np.float32)
    dinv[:nk] = np.where(dl > 0, 1.0 / np.maximum(dl, 1), 0.0)

    def build(keys, vals, scale, nwin, tw, width):
        order = np.argsort(keys, kind="stable")
        ks, vs, sc = keys[order], vals[order], scale[order]
        gidx = np.zeros(nwin * tw * 128, np.int16)
        widx = np.full(nwin * tw * 128, -1, np.int64)
        sedg = np.zeros(nwin * tw * 128, np.float32)
        starts = np.searchsorted(ks, np.arange(nwin) * width)
        ends = np.searchsorted(ks, (np.arange(nwin) + 1) * width)
        for w in range(nwin):
            a, b = starts[w], ends[w]
            n = b - a
            assert n <= tw * 128, f"window overflow {n} > {tw * 128}"
            o = w * tw * 128
            gidx[o:o + n] = vs[a:b]
            widx[o:o + n] = ks[a:b] - w * width
            sedg[o:o + n] = sc[a:b]
        return (_wrap_idx(gidx), _make_oh(widx, sedg, nwin * tw, bf16))

    # phase A: edges sorted by hedge; gather z rows (na); scale = binv[ha]
    giA, ohA = build(ha, na, binv[ha], NWA, TW_A, WA)
    # phase B: edges sorted by node; gather edge rows (ha); scale = dinv[na]
    giB, ohB = build(na, ha, dinv[na], NWB, TW_B, WB)

    xT = np.zeros((F_IN, NK_PAD), bf16)
    xT[:, :nk] = x[s:e].T.astype(bf16)

    p0 = np.zeros(NK_PAD, np.float32)
    p1 = np.zeros(NK_PAD, np.float32)
    bloc = batch[s:e]
    for gi, arr in ((2 * k, p0), (2 * k + 1, p1)):
        m = (bloc == gi)
        cnt = max(float(m.sum()), 1.0)
        arr[:nk][m] = 1.0 / cnt
    return {
        "xT": xT,
        "idxA": giA, "ohA": ohA,
        "idxB": giB, "ohB": ohB,
        "pool0_r": np.ascontiguousarray(
            np.broadcast_to(p0.astype(bf16), (128, NK_PAD))),
        "pool1_r": np.ascontiguousarray(
            np.broadcast_to(p1.astype(bf16), (128, NK_PAD))),
    }


def build_in_maps(x, node_idx, hedge_idx, batch, W0, b0, Wc1, bc1, Wc2, bc2,
                  Wg, bg, W1, b1, g1, be1, rm1, rv1, W2, b2, g2, be2, rm2,
                  rv2, W3):
    import ml_dtypes

    bf16 = ml_dtypes.bfloat16

    x = np.asarray(x, np.float32)
    node_idx = np.asarray(node_idx).astype(np.int64)
    hedge_idx = np.asarray(hedge_idx).astype(np.int64)
    batch_np = np.asarray(batch).astype(np.int64)

    # eval-BN folded into W1/W2
    k1 = np.asarray(g1) / np.sqrt(np.asarray(rv1) + EPS)
    W1f = (np.asarray(W1) * k1[None, :]).astype(np.float32)
    b1f = ((np.asarray(b1) - np.asarray(rm1)) * k1 + np.asarray(be1)).astype(
        np.float32)
    k2 = np.asarray(g2) / np.sqrt(np.asarray(rv2) + EPS)
    W2f = (np.asarray(W2) * k2[None, :]).astype(np.float32)
    b2f = ((np.asarray(b2) - np.asarray(rm2)) * k2 + np.asarray(be2)).astype(
        np.float32)

    cnt = np.bincount(hedge_idx, minlength=M_PAD).astype(np.float32)
    binv = np.where(cnt > 0, 1.0 / np.maximum(cnt, 1), 0.0).astype(np.float32)

    Wg_np = np.asarray(Wg, np.float32)
    shared = {
        "W0": np.asarray(W0, np.float32).astype(bf16),
        "Wc1": np.asarray(Wc1, np.float32).astype(bf16),
        "Wc2": np.asarray(Wc2, np.float32).astype(bf16),
        "WgA": Wg_np[:H], "WgB": Wg_np[H:],
        "W1f": W1f, "W2f": W2f, "W3": np.asarray(W3, np.float32),
        "b0c": np.asarray(b0, np.float32).reshape(-1, 1),
        "bc1c": np.asarray(bc1, np.float32).reshape(-1, 1),
        "bc2c": np.asarray(bc2, np.float32).reshape(-1, 1),
        "bgc": np.asarray(bg, np.float32).reshape(-1, 1),
        "b1c": b1f.reshape(-1, 1), "b2c": b2f.reshape(-1, 1),
    }
    in_maps = []
    for k in range(NCORE):
        m = _prep_core(k, x, node_idx, hedge_idx, batch_np, binv, bf16)
        m.update(shared)
        in_maps.append(m)
    return in_maps


def kernel(x, node_idx, hedge_idx, batch, W0, b0, Wc1, bc1, Wc2, bc2,
           Wg, bg, W1, b1, g1, be1, rm1, rv1, W2, b2, g2, be2, rm2, rv2, W3):
    global _COMPILED
    from concourse.bass_utils import run_bass_kernel_spmd

    if _COMPILED is None:
        _COMPILED = _build_nc()
    nc = _COMPILED

    in_maps = build_in_maps(
        x, node_idx, hedge_idx, batch, W0, b0, Wc1, bc1, Wc2, bc2, Wg, bg,
        W1, b1, g1, be1, rm1, rv1, W2, b2, g2, be2, rm2, rv2, W3)

    r = run_bass_kernel_spmd(nc, in_maps, core_ids=list(range(NCORE)))
    out = np.zeros((B_GRAPHS, 4), np.float32)
    for k in range(NCORE):
        o = r.results[k]["out"]
        out[2 * k] = o[:, 0]
        out[2 * k + 1] = o[:, 1]
    return out


# revision 16
# speedup vs baseline: 1.4430x; 1.3094x over previous
"""Trainium2 Bass kernel for the hypergraph-conv survival model.

Sharding: graph/data parallel over 8 NeuronCores. Core k owns graphs
2k,2k+1 (a contiguous node range, since `batch` is sorted). Both hconv
message-passing directions are sharded by that node range:
  phase A: gather local z rows by incidence sorted by hyperedge
           (SWDGE prepare_only gathers round-robined over 4 queues so
           DMA drains overlap); segment-sum via bf16 matmuls against
           host-precomputed one-hot tiles streamed from HBM. The
           one-hot is the stationary operand, so PSUM comes out
           hyperedge-major and needs no transpose; B^-1 is folded into
           the one-hot values per edge.
  AllReduce the [25088,128] bf16 partial hyperedge table (in 2 chunks,
           the first overlapping the tail of phase A).
  phase B: gather reduced hyperedge rows by incidence sorted by node;
           segment-sum into 128-wide node windows feature-major with
           D^-1 folded into the streamed one-hot; bias+relu on the Act
           engine; mean-pool weights applied on DVE.
Weights are replicated; the final MLP runs per-core on its two pooled
graph rows, all in fp32.
"""

import os
import sys

sys.path.insert(0, "/opt/trn_rl_repo")

import numpy as np

KDEBUG = bool(int(os.environ.get("KDEBUG", "0")))
NQ = int(os.environ.get("KNQ", "4"))        # SWDGE queues for gathers
SINGLE_PACKET = bool(int(os.environ.get("KSP", "0")))

# ---- static problem sizes (from the reference) ----
N = 100_000
E = 800_000
M = 25_000
B_GRAPHS = 16
F_IN = 64
H = 128
EPS = 1e-5
NCORE = 8

NK_PAD = 13312          # padded per-core node count (104*128)
M_PAD = 25088           # padded hyperedge count  (196*128)

WA = 128                # phase A window width (hyperedges)
NWA = M_PAD // WA       # 196 hedge windows
TW_A = 5                # tiles (128 edges) per hedge window (max seen 593)
NTA = NWA * TW_A        # 980 tiles
E_PAD_A = NTA * 128     # 125440
CHW_A = 14              # windows per gather chunk -> 14 chunks
NCH_A = NWA // CHW_A
NI_A = CHW_A * TW_A * 128   # idxs per phase-A gather (8960)

WB = 128                # phase B window width (nodes)
NWB = NK_PAD // WB      # 104 node windows
TW_B = 10               # tiles per node window (max seen 1165)
NTB = NWB * TW_B        # 1040 tiles
E_PAD_B = NTB * 128     # 133120
CHW_B = 8               # windows per gather chunk -> 13 chunks
NCH_B = NWB // CHW_B
NI_B = CHW_B * TW_B * 128   # idxs per phase-B gather (10240)

GT = 80                 # unified gather/one-hot chunk tile count (>=70,80)

AR_SPLIT_W = 98         # phase-A windows in the first AllReduce chunk
AR_SPLIT = AR_SPLIT_W * WA  # 12544 rows

_COMPILED = None


def _build_nc():
    import concourse.bacc as bacc
    import concourse.mybir as mybir
    from concourse.tile import TileContext
    from concourse import library_config

    f32 = mybir.dt.float32
    bf16 = mybir.dt.bfloat16
    i16 = mybir.dt.int16
    i32 = mybir.dt.int32
    EQ = mybir.AluOpType.is_equal
    ADD = mybir.AluOpType.add
    RELU = mybir.ActivationFunctionType.Relu

    nc = bacc.Bacc("TRN2", target_bir_lowering=False, num_devices=NCORE,
                   num_swdge_queues=NQ)

    def inp(name, shape, dt=f32):
        return nc.dram_tensor(name, shape, dt, kind="ExternalInput")

    xT = inp("xT", [F_IN, NK_PAD], bf16)
    idxA = inp("idxA", [128, E_PAD_A // 16], i16)
    idxB = inp("idxB", [128, E_PAD_B // 16], i16)
    ohA_d = inp("ohA", [128, NTA * 128], bf16)
    ohB_d = inp("ohB", [128, NTB * 128], bf16)
    pool0_r = inp("pool0_r", [128, NK_PAD], bf16)
    pool1_r = inp("pool1_r", [128, NK_PAD], bf16)
    W0_d = inp("W0", [F_IN, H], bf16)
    Wc1_d = inp("Wc1", [H, H], bf16)
    Wc2_d = inp("Wc2", [H, H], bf16)
    WgA_d = inp("WgA", [H, H])
    WgB_d = inp("WgB", [H, H])
    W1_d = inp("W1f", [H, 64])
    W2_d = inp("W2f", [64, 32])
    W3_d = inp("W3", [32, 4])
    b0_d = inp("b0c", [H, 1])
    bc1_d = inp("bc1c", [H, 1])
    bc2_d = inp("bc2c", [H, 1])
    bg_d = inp("bgc", [H, 1])
    b1_d = inp("b1c", [64, 1])
    b2_d = inp("b2c", [32, 1])
    out_d = nc.dram_tensor("out", [4, 2], f32, kind="ExternalOutput")

    dbg = {"kind": "ExternalOutput"} if KDEBUG else {}
    z1_h = nc.dram_tensor("z1_h", [NK_PAD, H], bf16, **dbg)
    z2_h = nc.dram_tensor("z2_h", [NK_PAD, H], bf16, **dbg)
    eA1 = nc.dram_tensor("eA1", [M_PAD, H], bf16)
    eR1 = nc.dram_tensor("eR1", [M_PAD, H], bf16, addr_space="Shared")
    eA2 = nc.dram_tensor("eA2", [M_PAD, H], bf16)
    eR2 = nc.dram_tensor("eR2", [M_PAD, H], bf16, addr_space="Shared")
    if KDEBUG:
        dumpR1 = nc.dram_tensor("dumpR1", [M_PAD, H], bf16, kind="ExternalOutput")
        dumpR2 = nc.dram_tensor("dumpR2", [M_PAD, H], bf16, kind="ExternalOutput")

    with TileContext(nc) as tc:
        with (
            tc.tile_pool(name="c", bufs=1) as cpool,
            tc.tile_pool(name="g", bufs=4) as gpool,
            tc.tile_pool(name="oh", bufs=2) as ohpool,
            tc.tile_pool(name="s", bufs=3) as spool,
            tc.tile_pool(name="ps", bufs=2, space="PSUM") as pspool,
            tc.tile_pool(name="pm", bufs=1, space="PSUM") as pmpool,
            tc.tile_pool(name="acc", bufs=1) as accpool,
            tc.tile_pool(name="bc", bufs=2) as bcpool,
        ):
            nc.gpsimd.load_library(library_config.mlp)
            sems = [nc.alloc_semaphore(f"swdge_dma{q}") for q in range(NQ)]
            uses = [0] * NQ
            gctr = [0]

            # ---- constants ----
            idn_i = cpool.tile([128, 128], i32)
            nc.gpsimd.iota(idn_i[:], [[1, 128]], channel_multiplier=-1)
            ident = cpool.tile([128, 128], bf16)
            nc.vector.tensor_scalar(ident[:], idn_i[:], 0.0, None, EQ)

            def load_sb(dram, shape, dt=f32):
                t = cpool.tile(shape, dt, tag=dram.name + "_sb")
                nc.sync.dma_start(out=t[:], in_=dram[:, :])
                return t

            W0s = load_sb(W0_d, [F_IN, H], bf16)
            Wc1s = load_sb(Wc1_d, [H, H], bf16)
            Wc2s = load_sb(Wc2_d, [H, H], bf16)
            WgAs = load_sb(WgA_d, [H, H])
            WgBs = load_sb(WgB_d, [H, H])
            W1s = load_sb(W1_d, [H, 64])
            W2s = load_sb(W2_d, [64, 32])
            W3s = load_sb(W3_d, [32, 4])
            b0s = load_sb(b0_d, [H, 1])
            bc1s = load_sb(bc1_d, [H, 1])
            bc2s = load_sb(bc2_d, [H, 1])
            bgs = load_sb(bg_d, [H, 1])
            b1s = load_sb(b1_d, [64, 1])
            b2s = load_sb(b2_d, [32, 1])
            idxAs = load_sb(idxA, [128, E_PAD_A // 16], i16)
            idxBs = load_sb(idxB, [128, E_PAD_B // 16], i16)

            p1acc = accpool.tile([128, 2], f32)
            nc.vector.memset(p1acc[:], 0.0)
            p2acc = accpool.tile([128, 2], f32)
            nc.vector.memset(p2acc[:], 0.0)

            def gather(table_h, idx_sb, i0, ni):
                """prepare_only gather of ni rows split over the SWDGE
                queues so the ring drains run in parallel; returns
                (tile, wait)."""
                g = gpool.tile([128, GT, 128], bf16, tag="g")
                nt = ni // 128
                waits = []
                for q in range(NQ):
                    t0 = q * nt // NQ
                    t1 = (q + 1) * nt // NQ
                    if t1 == t0:
                        continue
                    sub = (t1 - t0) * 128
                    nc.gpsimd.dma_gather(
                        g[:, t0:t1, :], table_h.ap(),
                        idx_sb[:, (i0 + t0 * 128) // 16:
                               (i0 + t1 * 128) // 16],
                        sub, sub, 128, single_packet=SINGLE_PACKET,
                        prepare_only=True, sem=sems[q], queue_num=q,
                    )
                    nc.gpsimd.trigger_dma(count=None, queue_num=q)
                    uses[q] += 1
                    waits.append((sems[q], 16 * uses[q]))

                def wait(eng):
                    for sem, target in waits:
                        eng.wait_ge(sem, target)
                return g, wait

            def wr_rows(zc, dst, row0):
                """Transpose [128f, 128n] bf16 block to node rows, DMA out."""
                tp = pspool.tile([128, 128], bf16, tag="trp")
                nc.tensor.transpose(tp[:], zc[:], ident[:])
                rows = spool.tile([128, 128], bf16, tag="rows")
                nc.scalar.copy(rows[:], tp[:])
                nc.sync.dma_start(out=dst[row0:row0 + 128, :], in_=rows[:])

            # ---- stage 1+2: z1 = (relu(x@W0+b0)) @ Wc1, node rows to HBM ----
            for c in range(NWB):
                xc = spool.tile([F_IN, WB], bf16, tag="xc")
                nc.sync.dma_start(out=xc[:], in_=xT[:, c * WB:(c + 1) * WB])
                ps1 = pspool.tile([128, WB], f32, tag="st")
                nc.tensor.matmul(ps1[:], W0s[:], xc[:], start=True, stop=True)
                h1c = spool.tile([128, WB], bf16, tag="h1c")
                nc.scalar.activation(h1c[:], ps1[:], RELU, bias=b0s[:, 0:1])
                ps2 = pspool.tile([128, WB], f32, tag="st")
                nc.tensor.matmul(ps2[:], Wc1s[:], h1c[:], start=True, stop=True)
                zc = spool.tile([128, WB], bf16, tag="zc")
                nc.scalar.copy(zc[:], ps2[:])
                wr_rows(zc, z1_h, c * WB)

            # ---- conv phase A: partial hyperedge sums (hedge-major PSUM) ----
            def phase_A(table_h, out_h, red_h):
                for ch in range(NCH_A):
                    g, gwait = gather(table_h, idxAs, ch * NI_A, NI_A)
                    ohs = ohpool.tile([128, GT, 128], bf16, tag="ohs")
                    nc.sync.dma_start(
                        out=ohs[:, :CHW_A * TW_A, :],
                        in_=ohA_d[:, ch * NI_A:(ch + 1) * NI_A])
                    gwait(nc.tensor)
                    for wi in range(CHW_A):
                        w = ch * CHW_A + wi
                        psw = pspool.tile([128, WB], f32, tag="ph")
                        ps = psw[:]
                        for tt in range(TW_A):
                            t = wi * TW_A + tt
                            nc.tensor.matmul(
                                ps, ohs[:, t, :], g[:, t, :],
                                start=(tt == 0), stop=(tt == TW_A - 1))
                        rows = spool.tile([128, 128], bf16, tag="rowsA")
                        nc.scalar.copy(rows[:], ps)
                        nc.sync.dma_start(
                            out=out_h[w * WA:(w + 1) * WA, :], in_=rows[:])
                    if ch == AR_SPLIT_W // CHW_A - 1:
                        nc.gpsimd.collective_compute(
                            "AllReduce", ADD,
                            replica_groups=[list(range(NCORE))],
                            ins=[out_h[0:AR_SPLIT, :].opt()],
                            outs=[red_h[0:AR_SPLIT, :].opt()])
                nc.gpsimd.collective_compute(
                    "AllReduce", ADD, replica_groups=[list(range(NCORE))],
                    ins=[out_h[AR_SPLIT:M_PAD, :].opt()],
                    outs=[red_h[AR_SPLIT:M_PAD, :].opt()])

            # ---- conv phase B: node sums + relu + pooling (+ next z) ----
            def phase_B(table_h, bias_s, pacc, Wnext, znext_h):
                for ch in range(NCH_B):
                    g, gwait = gather(table_h, idxBs, ch * NI_B, NI_B)
                    ohs = ohpool.tile([128, GT, 128], bf16, tag="ohs")
                    nc.sync.dma_start(
                        out=ohs[:, :CHW_B * TW_B, :],
                        in_=ohB_d[:, ch * NI_B:(ch + 1) * NI_B])
                    csl = slice(ch * CHW_B * WB, (ch + 1) * CHW_B * WB)
                    p0ch = bcpool.tile([128, CHW_B * WB], bf16, tag="p0ch")
                    nc.sync.dma_start(out=p0ch[:], in_=pool0_r[:, csl])
                    p1ch = bcpool.tile([128, CHW_B * WB], bf16, tag="p1ch")
                    nc.sync.dma_start(out=p1ch[:], in_=pool1_r[:, csl])
                    gwait(nc.tensor)
                    for wi in range(CHW_B):
                        w = ch * CHW_B + wi
                        psw = pspool.tile([128, WB], f32, tag="ph")
                        ps = psw[:]
                        for tt in range(TW_B):
                            t = wi * TW_B + tt
                            nc.tensor.matmul(
                                ps, g[:, t, :], ohs[:, t, :],
                                start=(tt == 0), stop=(tt == TW_B - 1))
                        h2c = spool.tile([128, WB], bf16, tag="h2c")
                        nc.scalar.activation(
                            h2c[:], ps, RELU, bias=bias_s[:, 0:1])
                        sl = slice(wi * WB, (wi + 1) * WB)
                        for gi, prow in ((0, p0ch), (1, p1ch)):
                            pm = spool.tile([128, WB], bf16, tag="pm")
                            nc.vector.tensor_mul(pm[:], h2c[:], prow[:, sl])
                            rs = spool.tile([128, 1], f32, tag="rs")
                            nc.vector.tensor_reduce(
                                rs[:], pm[:], mybir.AxisListType.X, ADD)
                            nc.vector.tensor_add(
                                pacc[:, gi:gi + 1], pacc[:, gi:gi + 1], rs[:])
                        if znext_h is not None:
                            zp = pspool.tile([128, WB], f32, tag="st")
                            nc.tensor.matmul(
                                zp[:], Wnext[:], h2c[:], start=True, stop=True)
                            zc = spool.tile([128, WB], bf16, tag="zc")
                            nc.scalar.copy(zc[:], zp[:])
                            wr_rows(zc, znext_h, w * WB)

            def dump_table(src_h, dst_h):
                for w in range(NWA):
                    t = spool.tile([128, H], bf16, tag="dmp")
                    nc.sync.dma_start(
                        out=t[:], in_=src_h[w * WA:(w + 1) * WA, :])
                    nc.sync.dma_start(
                        out=dst_h[w * WA:(w + 1) * WA, :], in_=t[:])

            phase_A(z1_h, eA1, eR1)
            if KDEBUG:
                dump_table(eR1, dumpR1)
            phase_B(eR1, bc1s, p1acc, Wc2s, z2_h)
            phase_A(z2_h, eA2, eR2)
            if KDEBUG:
                dump_table(eR2, dumpR2)
            phase_B(eR2, bc2s, p2acc, None, None)

            # ---- MLP head on the two pooled rows (fp32) ----
            gpsw = pmpool.tile([128, 2], f32, tag="mlp")
            gps = gpsw[:]
            nc.tensor.matmul(gps, WgAs[:], p1acc[:], start=True, stop=False)
            nc.tensor.matmul(gps, WgBs[:], p2acc[:], start=False, stop=True)
            gb = spool.tile([128, 2], f32, tag="m1")
            nc.vector.tensor_scalar(gb[:], gps, bgs[:, 0:1], None, ADD)
            h1psw = pmpool.tile([128, 2], f32, tag="mlp")
            h1ps = h1psw[:64, :]
            nc.tensor.matmul(h1ps, W1s[:], gb[:], start=True, stop=True)
            h1m = spool.tile([64, 2], f32, tag="m2")
            nc.vector.tensor_scalar(
                h1m[:], h1ps, b1s[:, 0:1], 0.0, ADD, mybir.AluOpType.max)
            h2psw = pmpool.tile([128, 2], f32, tag="mlp")
            h2ps = h2psw[:32, :]
            nc.tensor.matmul(h2ps, W2s[:], h1m[:], start=True, stop=True)
            h2m = spool.tile([32, 2], f32, tag="m3")
            nc.vector.tensor_scalar(
                h2m[:], h2ps, b2s[:, 0:1], 0.0, ADD, mybir.AluOpType.max)
            opsw = pmpool.tile([128, 2], f32, tag="mlp")
            ops = opsw[:4, :]
            nc.tensor.matmul(ops, W3s[:], h2m[:], start=True, stop=True)
            om = spool.tile([4, 2], f32, tag="m4")
            nc.vector.tensor_copy(om[:], ops)
            nc.sync.dma_start(out=out_d[:, :], in_=om[:])

    nc.compile()
    return nc


def _wrap_idx(idx):
    return np.tile(idx.reshape(-1, 16).T, (8, 1)).copy()


def _make_oh(widx, scale, ntiles, bf16):
    """[128, ntiles*128] bf16 one-hot tiles: partition=edge-in-tile,
    col block t holds tile t; oh[e, c] = scale[e] if widx[e]==c else 0."""
    flat = np.zeros((ntiles * 128, 128), np.float32)
    valid = widx >= 0
    rows = np.nonzero(valid)[0]
    flat[rows, widx[valid]] = scale[valid]
    return np.ascontiguousarray(
        flat.reshape(ntiles, 128, 128).transpose(1, 0, 2)
        .reshape(128, ntiles * 128)).astype(bf16)


def _prep_core(k, x, node_idx, hedge_idx, batch, binv, bf16):
    s = int(np.searchsorted(batch, 2 * k))
    e = int(np.searchsorted(batch, 2 * k + 2))
    nk = e - s
    assert nk <= NK_PAD
    sel = np.where((node_idx >= s) & (node_idx < e))[0]
    na = (node_idx[sel] - s).astype(np.int64)
    ha = hedge_idx[sel].astype(np.int64)

    deg = np.bincount(node_idx, minlength=N).astype(np.float32)
    dl = deg[s:e]
    dinv = np.zeros(NK_PAD, np.float32)
    dinv[:nk] = np.where(dl > 0, 1.0 / np.maximum(dl, 1), 0.0)

    def build(keys, vals, scale, nwin, tw, width):
        order = np.argsort(keys, kind="stable")
        ks, vs, sc = keys[order], vals[order], scale[order]
        gidx = np.zeros(nwin * tw * 128, np.int16)
        widx = np.full(nwin * tw * 128, -1, np.int64)
        sedg = np.zeros(nwin * tw * 128, np.float32)
        starts = np.searchsorted(ks, np.arange(nwin) * width)
        ends = np.searchsorted(ks, (np.arange(nwin) + 1) * width)
        for w in range(nwin):
            a, b = starts[w], ends[w]
            n = b - a
            assert n <= tw * 128, f"window overflow {n} > {tw * 128}"
            o = w * tw * 128
            gidx[o:o + n] = vs[a:b]
            widx[o:o + n] = ks[a:b] - w * width
            sedg[o:o + n] = sc[a:b]
        return (_wrap_idx(gidx), _make_oh(widx, sedg, nwin * tw, bf16))

    # phase A: edges sorted by hedge; gather z rows (na); scale = binv[ha]
    giA, ohA = build(ha, na, binv[ha], NWA, TW_A, WA)
    # phase B: edges sorted by node; gather edge rows (ha); scale = dinv[na]
    giB, ohB = build(na, ha, dinv[na], NWB, TW_B, WB)

    xT = np.zeros((F_IN, NK_PAD), bf16)
    xT[:, :nk] = x[s:e].T.astype(bf16)

    p0 = np.zeros(NK_PAD, np.float32)
    p1 = np.zeros(NK_PAD, np.float32)
    bloc = batch[s:e]
    for gi, arr in ((2 * k, p0), (2 * k + 1, p1)):
        m = (bloc == gi)
        cnt = max(float(m.sum()), 1.0)
        arr[:nk][m] = 1.0 / cnt
    return {
        "xT": xT,
        "idxA": giA, "ohA": ohA,
        "idxB": giB, "ohB": ohB,
        "pool0_r": np.ascontiguousarray(
            np.broadcast_to(p0.astype(bf16), (128, NK_PAD))),
        "pool1_r": np.ascontiguousarray(
            np.broadcast_to(p1.astype(bf16), (128, NK_PAD))),
    }


def build_in_maps(x, node_idx, hedge_idx, batch, W0, b0, Wc1, bc1, Wc2, bc2,
                  Wg, bg, W1, b1, g1, be1, rm1, rv1, W2, b2, g2, be2, rm2,
                  rv2, W3):
    import ml_dtypes

    bf16 = ml_dtypes.bfloat16

    x = np.asarray(x, np.float32)
    node_idx = np.asarray(node_idx).astype(np.int64)
    hedge_idx = np.asarray(hedge_idx).astype(np.int64)
    batch_np = np.asarray(batch).astype(np.int64)

    # eval-BN folded into W1/W2
    k1 = np.asarray(g1) / np.sqrt(np.asarray(rv1) + EPS)
    W1f = (np.asarray(W1) * k1[None, :]).astype(np.float32)
    b1f = ((np.asarray(b1) - np.asarray(rm1)) * k1 + np.asarray(be1)).astype(
        np.float32)
    k2 = np.asarray(g2) / np.sqrt(np.asarray(rv2) + EPS)
    W2f = (np.asarray(W2) * k2[None, :]).astype(np.float32)
    b2f = ((np.asarray(b2) - np.asarray(rm2)) * k2 + np.asarray(be2)).astype(
        np.float32)

    cnt = np.bincount(hedge_idx, minlength=M_PAD).astype(np.float32)
    binv = np.where(cnt > 0, 1.0 / np.maximum(cnt, 1), 0.0).astype(np.float32)

    Wg_np = np.asarray(Wg, np.float32)
    shared = {
        "W0": np.asarray(W0, np.float32).astype(bf16),
        "Wc1": np.asarray(Wc1, np.float32).astype(bf16),
        "Wc2": np.asarray(Wc2, np.float32).astype(bf16),
        "WgA": Wg_np[:H], "WgB": Wg_np[H:],
        "W1f": W1f, "W2f": W2f, "W3": np.asarray(W3, np.float32),
        "b0c": np.asarray(b0, np.float32).reshape(-1, 1),
        "bc1c": np.asarray(bc1, np.float32).reshape(-1, 1),
        "bc2c": np.asarray(bc2, np.float32).reshape(-1, 1),
        "bgc": np.asarray(bg, np.float32).reshape(-1, 1),
        "b1c": b1f.reshape(-1, 1), "b2c": b2f.reshape(-1, 1),
    }
    in_maps = []
    for k in range(NCORE):
        m = _prep_core(k, x, node_idx, hedge_idx, batch_np, binv, bf16)
        m.update(shared)
        in_maps.append(m)
    return in_maps


def kernel(x, node_idx, hedge_idx, batch, W0, b0, Wc1, bc1, Wc2, bc2,
           Wg, bg, W1, b1, g1, be1, rm1, rv1, W2, b2, g2, be2, rm2, rv2, W3):
    global _COMPILED
    from concourse.bass_utils import run_bass_kernel_spmd

    if _COMPILED is None:
        _COMPILED = _build_nc()
    nc = _COMPILED

    in_maps = build_in_maps(
        x, node_idx, hedge_idx, batch, W0, b0, Wc1, bc1, Wc2, bc2, Wg, bg,
        W1, b1, g1, be1, rm1, rv1, W2, b2, g2, be2, rm2, rv2, W3)

    r = run_bass_kernel_spmd(nc, in_maps, core_ids=list(range(NCORE)))
    out = np.zeros((B_GRAPHS, 4), np.float32)
    for k in range(NCORE):
        o = r.results[k]["out"]
        out[2 * k] = o[:, 0]
        out[2 * k + 1] = o[:, 1]
    return out
